# revision 5
# baseline (speedup 1.0000x reference)
"""Trainium2 Bass kernel for nn_Aggregation_Separation_Loss — v2 (trace design).

Math: pairwise SmoothL1 (beta=1, mean over D) for all (i,j):
    huber(z) = 0.5*z^2 - 0.5*relu(|z|-1)^2
    sl1[i,j]*D = 0.5*s_i + 0.5*s_j - G_ij - 0.5*V_ij
with V_ij = sum_d relu(|x_id-x_jd|-1)^2, and via relu(|z|-1)^2 =
relu(z-1)^2 + relu(-z-1)^2 only the one-sided P_ij = sum_d u^2,
u = relu(x_jd - x_id - 1), is needed over all ordered pairs.

Key identity (kills the elementwise square): with a_i = x_i + 1 and the
SHIFTED relu  û = max(x_j, a_i)  (so u = û - a_i):
    u^2 = (x_j - a_i)(û - a_i) = x_j·û - a_i·û - a_i·x_j + a_i^2
Summing over j in any column range R:
    sum_R u^2 = [sum_R x_j·û]      <- PE trace-matmul / DVE ttr (this core)
              - a_i·[sum_R û]      <- free accum_out of the û op / PE linear
              - a_i·[sum_R x_j]    <- host constant
              + |R|·||a_i||^2      <- host constant
The PE computes sum_R x_j·û via block matmuls lhsT=xt-block, rhs=û-block
accumulated into a single PSUM tile; only the DIAGONAL cells carry the
wanted per-column products (off-diagonal junk is ignored). So the only
elementwise pass left is the single DVE tensor_scalar (4x mode) that
produces û — the v1 ACT squares and PE selector matmuls are gone.

Masked (same-label) sums: columns are sorted by label, so each row's
same-label set is one contiguous column range. PE streams are SPLIT at
the range boundaries: in-range pieces accumulate into a second PSUM
tile (SA), the rest into SB; host adds both diagonals for the full sum.
The linear term over the range comes from 1-column-lhsT matmuls (lhsT =
a_i) accumulated into a third PSUM row, row-summed on host.

Load balance across engines (PE would otherwise be the bottleneck):
 - ACT_SLOTS slots produce u on the ACT engine (plain relu + accum).
 - TTR_SLOTS slots skip the PE trace entirely: DVE tensor_tensor_reduce
   computes sum x_j*û over the full row and over the label range into
   per-unit output columns.

Sharding: rows label-sorted and dealt round-robin (slot r of core c =
sorted row 8r+c) so every slot has the same label on all 8 cores except
the <=L-1 slots straddling a label boundary.  Those boundary slots
compute BOTH candidate ranges' sum u^2 directly (ACT produces true u,
DVE ttr squares the two ranges) into per-slot output columns; the host
picks the right candidate per core.

The Gram part (sum_same G, sum_all G) has a closed form in the label
count vectors and is evaluated on the host in f64, so the device only
computes the irreducible O(N^2 D) relu term.
"""

import os

import numpy as np

import concourse.bass as bass
import concourse.mybir as mybir
import concourse.tile as tile
from concourse.bacc import Bacc

N = 768
D = 256
NCORES = 8
SLOTS = 96
DT = 2
F32 = mybir.dt.float32
FP16 = mybir.dt.float16

EXTRA_ACT_SLOTS = 18             # non-boundary slots produced on ACT
TTK_SLOTS = 2                    # uni slots traced via DVE TT+TensorReduce
LINPE_SLOTS = 0                  # uni slots whose linear term runs on PE

_NC_CACHE = {}
BISECT = set(os.environ.get("K2_BISECT", "").split(","))


def layout_from_labels(lab):
    """All compile-time structure derived from the label vector."""
    lab = np.asarray(lab).astype(np.int64)
    order = np.argsort(lab, kind="stable")
    slab = lab[order]
    vals, starts, counts = np.unique(slab, return_index=True, return_counts=True)
    ranges = {int(v): (int(s), int(c)) for v, s, c in zip(vals, starts, counts)}
    # slot r of core c = sorted row order[8r + c]
    slot_rows = order.reshape(SLOTS, NCORES)  # [r, c] global row idx
    slot_labels = slab.reshape(SLOTS, NCORES)
    boundary = []
    slot_info = []
    for r in range(SLOTS):
        ls = np.unique(slot_labels[r])
        if len(ls) == 1:
            slot_info.append(("uni", int(ls[0])))
        else:
            assert len(ls) == 2, f"slot {r} spans {len(ls)} labels"
            slot_info.append(("bnd", int(ls[0]), int(ls[1])))
            boundary.append(r)
    # engine assignment: boundary slots on ACT; spread extra ACT + off-PE
    act_slots = set(boundary)
    near_bnd = set()
    for b in boundary:
        near_bnd.update((b - 2, b - 1, b + 1, b + 2))
    uni = [r for r in range(SLOTS)
           if r not in act_slots and r not in near_bnd]
    step = len(uni) / max(EXTRA_ACT_SLOTS, 1)
    for k in range(EXTRA_ACT_SLOTS):
        act_slots.add(uni[int(k * step + step / 2) % len(uni)])
    alluni = [r for r in range(SLOTS) if slot_info[r][0] == "uni"]
    ttr_slots = set()
    pool_slots = set()
    rest = [r for r in alluni if r not in act_slots]
    step = len(rest) / max(TTK_SLOTS, 1)
    ttk = set(rest[int(k * step + step / 2) % len(rest)]
              for k in range(TTK_SLOTS))
    ttr_idx = {r: i for i, r in enumerate(sorted(ttk))}
    linpe = set()
    if LINPE_SLOTS:
        step = len(alluni) / LINPE_SLOTS
        linpe = set(alluni[int(k * step + step / 2) % len(alluni)]
                    for k in range(LINPE_SLOTS))
    lind_units = [(r, t) for r in alluni if r not in linpe for t in range(2)]
    lind_idx = {u: i for i, u in enumerate(lind_units)}
    uni_units = [(r, t) for r in range(SLOTS)
                 if slot_info[r][0] == "uni" for t in range(2)]
    uni_idx = {u: i for i, u in enumerate(uni_units)}
    return dict(
        uni_units=uni_units,
        uni_idx=uni_idx,
        order=order,
        sorted_labels=slab,
        ranges=ranges,
        slot_rows=slot_rows,
        slot_info=slot_info,
        boundary=boundary,
        act_slots=act_slots,
        ttk=ttk,
        ttr_slots=ttr_slots,
        pool_slots=pool_slots,
        ttr_idx=ttr_idx,
        linpe=linpe,
        lind_units=lind_units,
        lind_idx=lind_idx,
    )


def _pieces(lo, hi, cuts):
    """Split [lo,hi) at the sorted cut points that fall inside."""
    pts = [lo] + [c for c in cuts if lo < c < hi] + [hi]
    return list(zip(pts[:-1], pts[1:]))


# fout column layout (all f32): [osb 128 | osa 128 | our 192 | tsb 2K | tsa 2K
#                                | obnd 4*NB | osl n_uni_units]
def fout_layout(L):
    off = {}
    off["osb"] = 0
    off["osa"] = 128
    off["our"] = 256
    off["tsb"] = 448
    off["tsa"] = 448 + 2 * len(L["ttk"])
    off["osl"] = 448 + 4 * len(L["ttk"])
    width = off["osl"] + len(L["lind_units"])
    return off, width


def build_nc(L):
    """L = layout dict from layout_from_labels."""
    nc = Bacc()
    off, FW = fout_layout(L)
    xtb_d = nc.dram_tensor("xtb", [128, DT * N], FP16, kind="ExternalInput")
    acp_d = nc.dram_tensor("acp", [128, DT * SLOTS], F32, kind="ExternalInput")
    acn_d = nc.dram_tensor("acn", [128, DT * SLOTS], F32, kind="ExternalInput")
    ach_d = nc.dram_tensor("ach", [128, DT * SLOTS], FP16, kind="ExternalInput")
    fout_d = nc.dram_tensor("fout", [128, FW], F32, kind="ExternalOutput")
    olin_d = nc.dram_tensor("olin", [1, 128], F32, kind="ExternalOutput")
    bidx = {r: k for k, r in enumerate(L["boundary"])}

    with tile.TileContext(nc) as tc:
        with (
            tc.tile_pool(name="pers", bufs=1) as pers,
            tc.tile_pool(name="us", bufs=10) as us,
            tc.tile_pool(name="sc", bufs=10) as sc,
            tc.tile_pool(name="psum", bufs=1, space=bass.MemorySpace.PSUM) as psum,
        ):
            xtb = pers.tile([128, DT * N], FP16, tag="xtb")
            acp = pers.tile([128, DT * SLOTS], F32, tag="acp")
            acn = pers.tile([128, DT * SLOTS], F32, tag="acn")
            ach = pers.tile([128, DT * SLOTS], FP16, tag="ach")
            fout = pers.tile([128, FW], F32, tag="fout")
            nc.gpsimd.dma_start(acp[:], acp_d[:])
            nc.gpsimd.dma_start(xtb[:, 0:N], xtb_d[:, 0:N])
            nc.gpsimd.dma_start(acn[:], acn_d[:])
            nc.gpsimd.dma_start(xtb[:, N : 2 * N], xtb_d[:, N : 2 * N])
            nc.gpsimd.dma_start(ach[:], ach_d[:])

            z = pers.tile([128, 128], FP16, tag="z")
            nc.vector.memset(z[:], 0.0)
            flin = pers.tile([1, 128], F32, tag="flin")
            if not L["linpe"]:
                nc.vector.memset(flin[:], 0.0)
                nc.gpsimd.dma_start(olin_d[:], flin[:])
            if BISECT and BISECT != {""}:
                nc.vector.memset(fout[:], 0.0)
            # dummy activation: loads the Relu table while DMAs are in flight
            zact = pers.tile([128, 1], FP16, tag="zact")
            nc.scalar.activation(
                zact[:], z[:, 0:1], mybir.ActivationFunctionType.Relu,
                bias=0.0, scale=1.0,
            )

            sb = psum.tile([128, 128], F32, tag="sb")
            sa = psum.tile([128, 128], F32, tag="sa")
            lin = psum.tile([1, 128], F32, tag="lin")
            # zero-init psum accumulators (also warms the PE p-state)
            nc.tensor.matmul(sb[:], z[:], z[:], start=True,
                             stop=False, skip_group_check=True)
            nc.tensor.matmul(sa[:], z[:], z[:], start=True,
                             stop=False, skip_group_check=True)
            nc.tensor.matmul(lin[:], z[:, 0:1], z[:], start=True,
                             stop=False, skip_group_check=True)

            def ucol(r, t):
                return fout[:, off["our"] + t * SLOTS + r :
                            off["our"] + t * SLOTS + r + 1]

            last_mm = {"sb": None, "sa": None}
            for r_ in range(SLOTS):
                if r_ in L["ttk"]:
                    continue
                info_ = L["slot_info"][r_]
                if info_[0] == "uni":
                    o_, c_ = L["ranges"][info_[1]]
                    cuts_ = (o_, o_ + c_)
                else:
                    cuts_ = ()
                for t_ in range(DT):
                    for b_ in range(6):
                        for s_, e_ in _pieces(128 * b_, 128 * (b_ + 1), cuts_):
                            key = ("sa" if cuts_ and cuts_[0] <= s_ < cuts_[1]
                                   else "sb")
                            last_mm[key] = (r_, t_, b_, s_, e_)
            for r in range(SLOTS):
                ust = us.tile([128, DT * N], FP16, tag="ust", name=f"ust_{r}")

                def useg(r_, t):
                    return ust[:, t * N : (t + 1) * N]

                info = L["slot_info"][r]
                on_act = r in L["act_slots"]
                # -- produce û (DVE, shifted) or u (ACT, plain) per unit --
                for t in range(DT):
                    col = t * SLOTS + r
                    if on_act:
                        if "noaccum" in BISECT:
                            nc.scalar.activation(
                                useg(r, t), xtb[:, t * N : (t + 1) * N],
                                mybir.ActivationFunctionType.Relu,
                                bias=acn[:, col : col + 1], scale=1.0,
                            )
                            nc.vector.memset(ucol(r, t), 0.0)
                        else:
                            nc.scalar.activation(
                                useg(r, t), xtb[:, t * N : (t + 1) * N],
                                mybir.ActivationFunctionType.Relu,
                                bias=acn[:, col : col + 1], scale=1.0,
                                accum_out=ucol(r, t),
                            )
                    elif "noaccum" in BISECT:
                        nc.vector.tensor_scalar(
                            useg(r, t), xtb[:, t * N : (t + 1) * N],
                            acp[:, col : col + 1], None,
                            op0=mybir.AluOpType.max,
                        )
                        nc.vector.memset(ucol(r, t), 0.0)
                    else:
                        nc.vector.tensor_scalar(
                            useg(r, t), xtb[:, t * N : (t + 1) * N],
                            acp[:, col : col + 1], 0.0,
                            op0=mybir.AluOpType.max,
                            op1=mybir.AluOpType.add,
                            accum_out=ucol(r, t),
                        )
                # ttk slots: v = xt*û on DVE, then free-axis reduces
                if r in L["ttk"]:
                    o, c = L["ranges"][info[1]]
                    for t in range(DT):
                        scv = sc.tile([128, N], FP16, tag="scv",
                                      name=f"scv_{r}_{t}")
                        nc.vector.tensor_tensor(
                            scv[:], xtb[:, t * N : (t + 1) * N],
                            useg(r, t)[:], op=mybir.AluOpType.mult,
                        )
                        kcol = off["tsb"] + 2 * L["ttr_idx"][r] + t
                        nc.vector.tensor_reduce(
                            fout[:, kcol : kcol + 1], scv[:],
                            axis=mybir.AxisListType.X, op=mybir.AluOpType.add,
                        )
                        kcol = off["tsa"] + 2 * L["ttr_idx"][r] + t
                        nc.vector.tensor_reduce(
                            fout[:, kcol : kcol + 1], scv[:, o : o + c],
                            axis=mybir.AxisListType.X, op=mybir.AluOpType.add,
                        )

                # -- PE stream for this slot --
                if info[0] == "uni" and "nosa" not in BISECT:
                    o, c = L["ranges"][info[1]]
                    cuts = (o, o + c)
                else:
                    cuts = ()
                for t in range(DT):
                    if r not in L["ttk"]:
                        for b in range(6):
                            lo, hi = 128 * b, 128 * (b + 1)
                            lw = xtb[:, t * N + lo : t * N + hi]
                            for s, e in _pieces(lo, hi, cuts):
                                dst, key = sb, "sb"
                                if cuts and cuts[0] <= s < cuts[1]:
                                    dst, key = sa, "sa"
                                nc.tensor.matmul(
                                    dst[:, s - lo : e - lo], lw,
                                    useg(r, t)[:, s:e],
                                    start=False,
                                    stop=(last_mm[key] == (r, t, b, s, e)),
                                    skip_group_check=True,
                                )
                    # linear term over the label range: PE 1-col matmuls for
                    # linpe slots, DVE tensor_scalar accum for the rest
                    if info[0] == "uni" and cuts:
                        col = t * SLOTS + r
                        if r in L["linpe"]:
                            for s, e in _pieces(cuts[0], cuts[1],
                                                (128, 256, 384, 512, 640)):
                                nc.tensor.matmul(
                                    lin[0:1, s % 128 : s % 128 + (e - s)],
                                    ach[:, col : col + 1],
                                    useg(r, t)[:, s:e],
                                    start=False, stop=False,
                                    skip_group_check=True,
                                )
                        else:
                            ocol = off["osl"] + L["lind_idx"][(r, t)]
                            scl = sc.tile([128, 128], FP16, tag="scl",
                                          name=f"scl_{r}_{t}")
                            nc.vector.tensor_scalar(
                                scl[:, 0 : cuts[1] - cuts[0]],
                                xtb[:, t * N + cuts[0] : t * N + cuts[1]],
                                acp[:, col : col + 1], 0.0,
                                op0=mybir.AluOpType.max,
                                op1=mybir.AluOpType.add,
                                accum_out=fout[:, ocol : ocol + 1],
                            )

            # close psum groups
            nc.tensor.matmul(lin[:], z[:, 0:1], z[:], start=False,
                             stop=True, skip_group_check=True)

            nc.gpsimd.dma_start(fout_d[:, 256:FW], fout[:, 256:FW])
            nc.vector.tensor_scalar(
                fout[:, off["osb"] : off["osb"] + 128], sb[:], 1.0, None,
                op0=mybir.AluOpType.mult,
            )
            nc.gpsimd.dma_start(fout_d[:, 0:128], fout[:, 0:128])
            nc.scalar.copy(fout[:, off["osa"] : off["osa"] + 128], sa[:])
            if L["linpe"]:
                nc.scalar.copy(flin[:], lin[:])
                nc.gpsimd.dma_start(olin_d[:], flin[:])
            nc.gpsimd.dma_start(fout_d[:, 128:256], fout[:, 128:256])

    nc.finalize()
    return nc


def prepare_inputs(X, lab, L):
    """Host-side tensors. X: [N, D] f32 -> per-core input dicts + host data."""
    order = L["order"]
    Xs = X[:, :]  # [N, D]
    XT = np.ascontiguousarray(Xs.T)[:, order]  # [D, 768] label-sorted cols
    xt16 = XT.astype(np.float16)
    xtb = np.empty((128, DT * N), np.float16)
    for t in range(DT):
        xtb[:, t * N : (t + 1) * N] = xt16[128 * t : 128 * (t + 1)]

    in_maps = []
    host = []
    for c in range(NCORES):
        rows = L["slot_rows"][:, c]  # global row index per slot
        # knee rounded to fp16 so device scalar == host constant exactly
        a16 = Xs[rows].T.astype(np.float64) + 1.0  # [D, 96]
        a16 = np.float16(a16).astype(np.float64)
        acp = np.empty((128, DT * SLOTS), np.float32)
        ach = np.empty((128, DT * SLOTS), np.float16)
        for t in range(DT):
            acp[:, t * SLOTS : (t + 1) * SLOTS] = a16[128 * t : 128 * (t + 1)]
            ach[:, t * SLOTS : (t + 1) * SLOTS] = a16[128 * t : 128 * (t + 1)]
        in_maps.append(dict(xtb=xtb, acp=acp, acn=-acp, ach=ach))
        slab_c = L["sorted_labels"][NCORES * np.arange(SLOTS) + c]
        host.append(dict(rows=rows, a16=a16, slab=slab_c))
    return in_maps, host, xt16


def host_finish_core(L, hostc, xt16, fout, olin):
    """Per-core SA_P, SB_P (one-sided relu^2 sums) in f64 from device out."""
    off, FW = fout_layout(L)
    fout = np.asarray(fout, np.float64)
    olin = np.asarray(olin, np.float64).ravel()
    a16 = hostc["a16"]  # [D, 96] f64 (fp16-exact)
    xt = xt16.astype(np.float64)  # [D, 768]
    SXfull = xt.sum(axis=1)  # [D]
    bidx = {r: k for k, r in enumerate(L["boundary"])}

    osb = fout[:, off["osb"] : off["osb"] + 128]
    osa = fout[:, off["osa"] : off["osa"] + 128]
    our = fout[:, off["our"] : off["our"] + DT * SLOTS]
    osl = fout[:, off["osl"] : off["osl"] + len(L["lind_units"])]
    tsb = fout[:, off["tsb"] : off["tsa"]]
    tsa = fout[:, off["tsa"] : off["osl"]]

    SB = float(np.trace(osb) + np.trace(osa)) + float(tsb.sum())
    SA = float(np.trace(osa)) + float(tsa.sum()) - float(olin.sum())
    for (r, t), i in L["lind_idx"].items():
        a = a16[128 * t : 128 * (t + 1), r]
        SA -= float(a @ osl[:, i])
        if r in L["act_slots"]:
            # osl sums û=max(xt,a) but the ACT unit streamed u=û-a:
            # a·sum u = a·sum û - c·||a||^2
            _, cc = L["ranges"][L["slot_info"][r][1]]
            SA += cc * float(a @ a)
    for r in range(SLOTS):
        info = L["slot_info"][r]
        on_act = r in L["act_slots"]
        for t in range(DT):
            a = a16[128 * t : 128 * (t + 1), r]  # [128]
            U = our[:, t * SLOTS + r]  # sum_j û (or u for act units)
            SB -= float(a @ U)
            if not on_act:
                SB -= float(a @ SXfull[128 * t : 128 * (t + 1)]) - N * float(a @ a)
        if info[0] == "uni":
            if not on_act:
                o, cc = L["ranges"][info[1]]
                for t in range(DT):
                    a = a16[128 * t : 128 * (t + 1), r]
                    SXsl = xt[128 * t : 128 * (t + 1), o : o + cc].sum(axis=1)
                    SA -= float(a @ SXsl) - cc * float(a @ a)
        else:
            # boundary slot: this row's same-label slice sum computed on the
            # host in f64 (<=1% of the pairwise work)
            o, cc = L["ranges"][int(hostc["slab"][r])]
            u = np.maximum(xt[:, o : o + cc] - a16[:, r : r + 1], 0.0)
            SA += float((u * u).sum())
    return SA, SB


def gram_host(X, lab):
    """Closed-form sums of the Gram part in f64."""
    Xd = np.asarray(X, np.float64)
    lab = np.asarray(lab)
    tot = Xd.sum(axis=0)
    SG_all = float(tot @ tot)
    SG_same = 0.0
    for l in np.unique(lab):
        m = Xd[lab == l].sum(axis=0)
        SG_same += float(m @ m)
    return SG_same, SG_all


def host_finish(X, lab, SA, SB):
    """Combine totals into the three losses (same algebra as v1)."""
    Xd = X.astype(np.float64)
    s = (Xd * Xd).sum(axis=1)
    Ssum = s.sum()
    labs, counts = np.unique(lab, return_counts=True)
    Sl = np.array([s[lab == l].sum() for l in labs])
    n1 = int((counts.astype(np.int64) ** 2).sum())
    n2 = N * N - n1

    inner_sum = ((counts * Sl).sum() - SA) / D
    total_sum = (N * Ssum - SB) / D
    outer_sum = total_sum - inner_sum

    loss_inner = inner_sum / n1 if n1 > 0 else inner_sum
    loss_outer = outer_sum / max(n2, 1) if n2 > 0 else outer_sum
    penalty = ((np.sqrt(s) - 10.0) ** 2).mean()
    return (
        np.float32(loss_inner),
        np.float32(loss_outer),
        np.float32(penalty),
    )


def kernel(distributions, labels):
    from concourse.bass_utils import run_bass_kernel_spmd

    X = np.asarray(distributions, dtype=np.float32)
    lab = np.asarray(labels).astype(np.int64)
    assert X.shape == (N, D), X.shape

    L = layout_from_labels(lab)
    key = L["sorted_labels"].tobytes()
    if key not in _NC_CACHE:
        _NC_CACHE[key] = build_nc(L)
    nc = _NC_CACHE[key]

    in_maps, host, xt16 = prepare_inputs(X, lab, L)
    results = run_bass_kernel_spmd(nc, in_maps, list(range(NCORES))).results
    SA = 0.0
    SB = 0.0
    for c in range(NCORES):
        sa_c, sb_c = host_finish_core(L, host[c], xt16, results[c]["fout"],
                                      results[c]["olin"])
        SA += sa_c
        SB += sb_c
    SG_same, SG_all = gram_host(X, lab)
    return host_finish(X, lab, SA + SG_same, SB + SG_all)
